# revision 8
# baseline (speedup 1.0000x reference)
"""Trainium2 Bass kernel for nn_Decoder (GRU decoder over padded sequences).

Computation (per sample):
  emb = message[:, :T-1] @ W_emb.T + b_emb            (folded into W_c on host)
  xs  = [init_emb, emb]                                (step 0 folded into h1 const)
  GRU over T steps, gather h at lengths-1              (freeze trick: z := 1 past len)
  out = sigmoid(elu(h @ W1.T + b1) @ W2.T + b2)

Sharding: batch data-parallel over 8 cores, host-side length-sort (stratified
round-robin across cores) so each duo only runs to its max length.

DUO layout: every on-chip tile stacks TWO 512-sample blocks on the partition
axis (block-lo on partitions 0:63, block-hi on 64:127).  Elementwise cost on
DVE/ACT scales with the free dim only, so running the GRU's 64-feature math
at 128 partitions halves the per-sample elementwise cost vs a [64, FD]
layout.  Matmuls use PE quadrant tiling: lo contracts array rows 0:63 into
cols 0:63, hi contracts rows 64:127 into cols 64:127 (weights duplicated in
both halves), so lo/hi matmuls execute concurrently in disjoint quadrants.
Gate biases ride ones-rows of the x tile; b_hn is applied by the DVE
scalar_tensor_tensor that forms r*(hn+b_hn).
"""

import sys

sys.path.insert(0, "/opt/trn_rl_repo")

import numpy as np
import ml_dtypes

import concourse.bacc as bacc
import concourse.mybir as mybir
import concourse.tile as tile
from concourse.bass_utils import run_bass_kernel_spmd

B, T, V, E, H, FC, OUT = 65536, 30, 21, 32, 64, 256, 784
NCORES = 8
BC = B // NCORES          # 8192 samples per core
BLK = 512                 # one sample block (one PSUM bank of fp32)
DUO = 2 * BLK             # samples per duo (lo block + hi block)
ND = BC // DUO            # 8 duos per core
FREEZE = 40.0             # z-gate preactivation offset for finished samples
KX = V + 1                # 21 msg rows + 1 freeze-flag row (DMA'd)
KXR = 33                  # x contraction: rows 0:22 data, 22:32 zero, 32 ones
OTILES = (OUT + 127) // 128  # 7 output row tiles

USE_BF16 = True
GP_U = True               # u = h - n on GpSimd (else VectorE)
GP_HP = True              # h' = n + v on GpSimd
GP_FR = True              # fr = pp + e on GpSimd
MLP_INLINE = True         # emit each 2-duo MLP group as soon as its duos finish
TRACE = False             # set by test harness for profiling
LAST_RESULT = None        # BassKernelResults stash for the harness

_f32 = mybir.dt.float32
_bf16 = mybir.dt.bfloat16


def _sigmoid(x):
    return 1.0 / (1.0 + np.exp(-x))


def _build_nc(duo_steps, dt):
    """Build the SPMD kernel. duo_steps[d] = GRU steps (beyond the constant
    step 0) for duo d — identical on every core."""
    AF = mybir.ActivationFunctionType
    OP = mybir.AluOpType
    nc = bacc.Bacc("TRN2", target_bir_lowering=False, debug=False)

    X = nc.dram_tensor("X", [T - 1, KX, BC], dt, kind="ExternalInput")
    WX = nc.dram_tensor("WX", [128, 3 * H], dt, kind="ExternalInput")
    WH = nc.dram_tensor("WH", [128, 3 * H], dt, kind="ExternalInput")
    W1D = nc.dram_tensor("W1D", [128, FC], dt, kind="ExternalInput")
    W2T = nc.dram_tensor("W2T", [FC, OUT], dt, kind="ExternalInput")
    B1 = nc.dram_tensor("B1", [128, FC // 128], _f32, kind="ExternalInput")
    B2 = nc.dram_tensor("B2", [128, OTILES], _f32, kind="ExternalInput")
    H1D = nc.dram_tensor("H1D", [128, 1], _f32, kind="ExternalInput")
    BHN = nc.dram_tensor("BHN", [128, 1], _f32, kind="ExternalInput")
    OT = nc.dram_tensor("OT", [OUT, BC], _f32, kind="ExternalOutput")

    ew_u = nc.gpsimd if GP_U else nc.vector
    ew_hp = nc.gpsimd if GP_HP else nc.vector
    ew_fr = nc.gpsimd if GP_FR else nc.vector

    with tile.TileContext(nc) as tc:
        with (
            tc.tile_pool(name="weights", bufs=1) as wp,
            tc.tile_pool(name="xin", bufs=8) as xp,
            tc.tile_pool(name="hstate", bufs=ND) as hp,
            tc.tile_pool(name="hfinal", bufs=1) as hf,
            tc.tile_pool(name="gates", bufs=3) as gp,
            tc.tile_pool(name="head", bufs=3) as fp,
            tc.tile_pool(name="frhs", bufs=1) as frp,
            tc.tile_pool(name="outs", bufs=3) as op_,
            tc.tile_pool(name="psA", bufs=2, space="PSUM") as psA,
            tc.tile_pool(name="psB", bufs=2, space="PSUM") as psB,
        ):
            # --- load weights/biases once (host pre-packs duo duplication) ---
            wx = wp.tile([128, 3 * H], dt)
            nc.sync.dma_start(out=wx[:], in_=WX[:])
            wh = wp.tile([128, 3 * H], dt)
            nc.sync.dma_start(out=wh[:], in_=WH[:])
            w1d = wp.tile([128, FC], dt)
            nc.sync.dma_start(out=w1d[:], in_=W1D[:])
            w2a = wp.tile([128, OUT], dt)
            nc.sync.dma_start(out=w2a[:], in_=W2T[0:128, :])
            w2b = wp.tile([128, OUT], dt)
            nc.sync.dma_start(out=w2b[:], in_=W2T[128:256, :])
            b1s = wp.tile([128, FC // 128], _f32)
            nc.sync.dma_start(out=b1s[:], in_=B1[:])
            b2s = wp.tile([128, OTILES], _f32)
            nc.sync.dma_start(out=b2s[:], in_=B2[:])
            h1d = wp.tile([128, 1], _f32)
            nc.sync.dma_start(out=h1d[:], in_=H1D[:])
            bhn = wp.tile([128, 1], _f32)
            nc.sync.dma_start(out=bhn[:], in_=BHN[:])

            # x tiles: ones rows (bias folding) preset once per pool buffer;
            # the per-step DMA only touches rows 0:KX and 64:64+KX.
            for _xi in range(8):
                xz = xp.tile([128, BLK], dt, tag="xt")
                nc.vector.memset(xz[:], 0.0)
                nc.vector.memset(xz[32:33, :], 1.0)
                nc.vector.memset(xz[96:97, :], 1.0)

            def start_duo(d):
                ha = hp.tile([128, BLK], dt, tag="ha")
                hb = hp.tile([128, BLK], dt, tag="hb")
                hfd = hf.tile([128, BLK], dt, tag=f"hf{d}")
                nc.vector.memset(ha[:], 0.0)
                nc.vector.tensor_scalar_add(ha[:], ha[:], h1d[:])
                if duo_steps[d] == 0:
                    nc.vector.memset(hfd[:], 0.0)
                    nc.vector.tensor_scalar_add(hfd[:], hfd[:], h1d[:])
                return {"cur": ha, "nxt": hb, "hf": hfd, "d": d}

            def emit_step(st, s):
                d = st["d"]
                cur = st["cur"]
                xt = xp.tile([128, BLK], dt, tag="xt")
                lo = d * DUO
                hi = lo + BLK
                nc.sync.dma_start(out=xt[0:KX, :], in_=X[s - 1, :, lo : lo + BLK])
                nc.sync.dma_start(
                    out=xt[64 : 64 + KX, :], in_=X[s - 1, :, hi : hi + BLK]
                )
                prz = psA.tile([128, DUO], _f32, tag="rz")
                pn = psB.tile([128, DUO], _f32, tag="n")
                # gate matmuls; weight cols r=0:64, z=64:128, n=128:192.
                # lo quadrant (rows 0:64 -> cols 0:64), hi (64:128 -> 64:128);
                # adjacent lo/hi matmuls execute concurrently.
                for gc, fds in ((0, slice(0, BLK)), (64, slice(BLK, DUO))):
                    # r (gc=0) / z (gc=64) preactivations -> prz[:, fds]
                    nc.tensor.matmul(
                        prz[0:64, fds], wh[0:64, gc : gc + 64], cur[0:64, :],
                        start=True, stop=False,
                    )
                    nc.tensor.matmul(
                        prz[64:128, fds], wh[64:128, gc : gc + 64], cur[64:128, :],
                        start=True, stop=False,
                    )
                    nc.tensor.matmul(
                        prz[0:64, fds], wx[0:KXR, gc : gc + 64], xt[0:KXR, :],
                        start=False, stop=True,
                    )
                    nc.tensor.matmul(
                        prz[64:128, fds], wx[64 : 64 + KXR, gc : gc + 64],
                        xt[64 : 64 + KXR, :], start=False, stop=True,
                    )
                # hn -> pn[:, 0:BLK] (h side only), xn -> pn[:, BLK:] (x side)
                nc.tensor.matmul(
                    pn[0:64, 0:BLK], wh[0:64, 128:192], cur[0:64, :],
                    start=True, stop=True,
                )
                nc.tensor.matmul(
                    pn[64:128, 0:BLK], wh[64:128, 128:192], cur[64:128, :],
                    start=True, stop=True,
                )
                nc.tensor.matmul(
                    pn[0:64, BLK:], wx[0:KXR, 128:192], xt[0:KXR, :],
                    start=True, stop=True,
                )
                nc.tensor.matmul(
                    pn[64:128, BLK:], wx[64 : 64 + KXR, 128:192],
                    xt[64 : 64 + KXR, :], start=True, stop=True,
                )
                # elementwise (all duo-width: 128 partitions, FD 512/1024)
                rz = gp.tile([128, DUO], dt, tag="rz")
                nc.scalar.activation(rz[:], prz[:], AF.Sigmoid)
                t1 = gp.tile([128, BLK], dt, tag="t1")
                nc.vector.scalar_tensor_tensor(
                    t1[:], pn[:, 0:BLK], bhn[:], rz[:, 0:BLK],
                    op0=OP.add, op1=OP.mult,
                )
                t2 = gp.tile([128, BLK], dt, tag="t2")
                nc.vector.tensor_add(t2[:], pn[:, BLK:], t1[:])
                nt = gp.tile([128, BLK], dt, tag="nt")
                nc.scalar.activation(nt[:], t2[:], AF.Tanh)
                u = gp.tile([128, BLK], dt, tag="u")
                ew_u.tensor_sub(u[:], cur[:], nt[:])
                v = gp.tile([128, BLK], dt, tag="v")
                nc.vector.tensor_mul(v[:], rz[:, BLK:], u[:])
                dst = st["hf"] if s == duo_steps[d] else st["nxt"]
                ew_hp.tensor_add(dst[:], nt[:], v[:])
                st["cur"], st["nxt"] = dst, st["cur"]

            def emit_mlp(g, hfin):
                """fc1 + ELU + fc2 + sigmoid for 2-duo group g."""
                d0, d1 = 2 * g, 2 * g + 1
                frs = {}
                for j in range(FC // 128):
                    for half, base in ((0, 0), (1, 64)):
                        pf = psA.tile([128, DUO], _f32, tag="rz")
                        for k, dd in enumerate((d0, d1)):
                            nc.tensor.matmul(
                                pf[:, k * BLK : (k + 1) * BLK],
                                w1d[base : base + 64, j * 128 : (j + 1) * 128],
                                hfin[dd][base : base + 64, :],
                                start=True, stop=True,
                            )
                        # elu(x+b1) + 1 == (x+b1 - m) + exp(m),  m = min(x+b1, 0)
                        m = fp.tile([128, DUO], dt, tag="m")
                        nc.vector.tensor_scalar(
                            m[:], pf[:], b1s[:, j : j + 1], 0.0,
                            op0=OP.add, op1=OP.min,
                        )
                        e = fp.tile([128, DUO], dt, tag="e")
                        nc.scalar.activation(e[:], m[:], AF.Exp)
                        pp = fp.tile([128, DUO], dt, tag="pp")
                        nc.vector.scalar_tensor_tensor(
                            pp[:], pf[:], b1s[:, j : j + 1], m[:],
                            op0=OP.add, op1=OP.subtract,
                        )
                        fr = frp.tile([128, DUO], dt, tag=f"fr{g}{j}{half}")
                        ew_fr.tensor_add(fr[:], pp[:], e[:])
                        frs[(j, half)] = fr
                for half in (0, 1):
                    for mt in range(OTILES):
                        mw = min(128, OUT - mt * 128)
                        po = psB.tile([128, DUO], _f32, tag="n")
                        for cs in (slice(0, BLK), slice(BLK, DUO)):
                            nc.tensor.matmul(
                                po[0:mw, cs], w2a[:, mt * 128 : mt * 128 + mw],
                                frs[(0, half)][:, cs], start=True, stop=False,
                            )
                            nc.tensor.matmul(
                                po[0:mw, cs], w2b[:, mt * 128 : mt * 128 + mw],
                                frs[(1, half)][:, cs], start=False, stop=True,
                            )
                        ot = op_.tile([mw, DUO], _f32, tag="ot")
                        nc.scalar.activation(
                            ot[:], po[0:mw, :], AF.Sigmoid,
                            bias=b2s[0:mw, mt : mt + 1],
                        )
                        # group g = blocks 4g..4g+3; half-lo = blocks 4g, 4g+2
                        b0 = (4 * g + half) * BLK
                        b1_ = (4 * g + 2 + half) * BLK
                        nc.sync.dma_start(
                            out=OT[mt * 128 : mt * 128 + mw, b0 : b0 + BLK],
                            in_=ot[:, 0:BLK],
                        )
                        nc.sync.dma_start(
                            out=OT[mt * 128 : mt * 128 + mw, b1_ : b1_ + BLK],
                            in_=ot[:, BLK:],
                        )

            states = [start_duo(d) for d in range(ND)]
            hfin = [st["hf"] for st in states]
            maxs = max(duo_steps)
            done_g = set()
            for s in range(1, maxs + 1):
                for st in states:
                    if s <= duo_steps[st["d"]]:
                        emit_step(st, s)
                if MLP_INLINE:
                    for g in range(ND // 2):
                        if g not in done_g and duo_steps[2 * g + 1] <= s:
                            emit_mlp(g, hfin)
                            done_g.add(g)
            if not MLP_INLINE:
                tc.no_sync_barrier()
            for g in range(ND // 2):
                if g not in done_g:
                    emit_mlp(g, hfin)

    nc.compile()
    return nc


def kernel(message, lengths, init_emb, W_emb, b_emb, W_ih, W_hh, b_ih, b_hh,
           W1, b1, W2, b2):
    global LAST_RESULT
    message = np.asarray(message, dtype=np.float32)
    lengths = np.asarray(lengths).astype(np.int64)
    f8 = np.float64
    np_dt = ml_dtypes.bfloat16 if USE_BF16 else np.float32
    dt = _bf16 if USE_BF16 else _f32

    # --- fold embedding into input weights;  step 0 is a constant ---
    W_c = W_ih.astype(f8) @ W_emb.astype(f8)                # [3H, V]
    b_c = W_ih.astype(f8) @ b_emb.astype(f8) + b_ih         # [3H]
    gx0 = W_ih.astype(f8) @ init_emb.astype(f8) + b_ih
    gh0 = b_hh.astype(f8)
    r0 = _sigmoid(gx0[:H] + gh0[:H])
    z0 = _sigmoid(gx0[H : 2 * H] + gh0[H : 2 * H])
    n0 = np.tanh(gx0[2 * H :] + r0 * gh0[2 * H :])
    h1 = (1.0 - z0) * n0                                    # h after step 0

    # --- length-sort, stratify across cores ---
    perm = np.argsort(lengths, kind="stable")
    lsort = lengths[perm]
    # duo d (on every core) covers global sorted ranks [d*8192, (d+1)*8192)
    duo_steps = [int(lsort[min((d + 1) * DUO * NCORES, B) - 1]) - 1
                 for d in range(ND)]

    # --- shared weight tensors (duo: both partition halves identical) ---
    brz = (b_c[: 2 * H] + b_hh[: 2 * H])
    WXh = np.zeros((64, 3 * H), f8)
    WXh[:V] = W_c.T
    WXh[V, H : 2 * H] = FREEZE
    WXh[32, : 2 * H] = brz
    WXh[32, 2 * H :] = b_c[2 * H :]
    wxd = np.concatenate([WXh, WXh]).astype(np_dt)
    whh = np.ascontiguousarray(W_hh.T).astype(f8)
    whd = np.concatenate([whh, whh]).astype(np_dt)
    w1h = np.ascontiguousarray(W1.T).astype(f8)
    w1dd = np.concatenate([w1h, w1h]).astype(np_dt)
    w2d = np.ascontiguousarray(W2.T).astype(np_dt)
    b1d = np.ascontiguousarray(np.asarray(b1, np.float32).reshape(FC // 128, 128).T)
    b2f = (np.asarray(b2, f8) - W2.astype(f8).sum(axis=1)).astype(np.float32)
    b2p = np.zeros(OTILES * 128, np.float32)
    b2p[:OUT] = b2f
    b2d = np.ascontiguousarray(b2p.reshape(OTILES, 128).T)
    h1d = np.ascontiguousarray(
        np.concatenate([h1, h1]).astype(np.float32).reshape(128, 1))
    bhnd = np.ascontiguousarray(
        np.concatenate([b_hh[2 * H :], b_hh[2 * H :]]).astype(np.float32)
        .reshape(128, 1))

    # --- per-core inputs ---
    trange = np.arange(T - 1)
    in_maps = []
    core_idx = []
    for c in range(NCORES):
        ic = perm[c::NCORES]
        core_idx.append(ic)
        mc = message[ic][:, : T - 1, :]                     # [BC, 29, 21]
        Xc = np.empty((T - 1, KX, BC), dtype=np_dt)
        Xc[:, :V, :] = mc.transpose(1, 2, 0).astype(np_dt)
        Xc[:, V, :] = (lengths[ic][None, :] <= trange[:, None] + 1).astype(np_dt)
        in_maps.append({
            "X": Xc, "WX": wxd, "WH": whd, "W1D": w1dd, "W2T": w2d,
            "B1": b1d, "B2": b2d, "H1D": h1d, "BHN": bhnd,
        })

    nc = _build_nc(duo_steps, dt)
    res = run_bass_kernel_spmd(nc, in_maps, core_ids=list(range(NCORES)), trace=TRACE)
    LAST_RESULT = res

    out = np.empty((B, OUT), np.float32)
    for c in range(NCORES):
        out[core_idx[c]] = res.results[c]["OT"].T
    return out


# revision 11
# speedup vs baseline: 1.1060x; 1.1060x over previous
"""Trainium2 Bass kernel for nn_Decoder (GRU decoder over padded sequences).

Computation (per sample):
  emb = message[:, :T-1] @ W_emb.T + b_emb            (folded into W_c on host)
  xs  = [init_emb, emb]                                (step 0 folded into h1 const)
  GRU over T steps, gather h at lengths-1              (freeze trick: z := 1 past len)
  out = sigmoid(elu(h @ W1.T + b1) @ W2.T + b2)

Sharding: batch data-parallel over 8 cores, host-side length-sort (stratified
round-robin across cores) so each duo only runs to its max length.

DUO layout: every on-chip tile stacks TWO 512-sample blocks on the partition
axis (block-lo on partitions 0:63, block-hi on 64:127).  Elementwise cost on
DVE/ACT scales with the free dim only, so running the GRU's 64-feature math
at 128 partitions halves the per-sample elementwise cost vs a [64, FD]
layout.  Matmuls use PE quadrant tiling: lo contracts array rows 0:63 into
cols 0:63, hi contracts rows 64:127 into cols 64:127 (weights duplicated in
both halves), so lo/hi matmuls execute concurrently in disjoint quadrants.
Gate biases ride ones-rows of the x tile; b_hn is applied by the DVE
scalar_tensor_tensor that forms r*(hn+b_hn).
"""

import sys

sys.path.insert(0, "/opt/trn_rl_repo")

import numpy as np
import ml_dtypes

import concourse.bacc as bacc
import concourse.mybir as mybir
import concourse.tile as tile
from concourse.bass_utils import run_bass_kernel_spmd

B, T, V, E, H, FC, OUT = 65536, 30, 21, 32, 64, 256, 784
NCORES = 8
BC = B // NCORES          # 8192 samples per core
BLK = 512                 # one sample block (one PSUM bank of fp32)
DUO = 2 * BLK             # samples per duo (lo block + hi block)
ND = BC // DUO            # 8 duos per core
FREEZE = 40.0             # z-gate preactivation offset for finished samples
KX = V + 1                # 21 msg rows + 1 freeze-flag row (DMA'd)
KXR = 33                  # x contraction: rows 0:22 data, 22:32 zero, 32 ones
OTILES = (OUT + 127) // 128  # 7 output row tiles

USE_BF16 = True
GP_U = True               # u = h - n on GpSimd (else VectorE)
GP_HP = False             # h' = n + v on GpSimd
GP_FR = True              # fr = pp + e on GpSimd
MLP_INLINE = True         # emit each 2-duo MLP group as soon as its duos finish
TRACE = False             # set by test harness for profiling
LAST_RESULT = None        # BassKernelResults stash for the harness

_f32 = mybir.dt.float32
_bf16 = mybir.dt.bfloat16


def _sigmoid(x):
    return 1.0 / (1.0 + np.exp(-x))


def _build_nc(duo_steps, dt):
    """Build the SPMD kernel. duo_steps[d] = GRU steps (beyond the constant
    step 0) for duo d — identical on every core."""
    AF = mybir.ActivationFunctionType
    OP = mybir.AluOpType
    nc = bacc.Bacc("TRN2", target_bir_lowering=False, debug=False)

    X = nc.dram_tensor("X", [T - 1, KX, BC], dt, kind="ExternalInput")
    WX = nc.dram_tensor("WX", [128, 3 * H], dt, kind="ExternalInput")
    WH = nc.dram_tensor("WH", [128, 3 * H], dt, kind="ExternalInput")
    W1D = nc.dram_tensor("W1D", [128, FC], dt, kind="ExternalInput")
    W2T = nc.dram_tensor("W2T", [FC, OUT], dt, kind="ExternalInput")
    B1 = nc.dram_tensor("B1", [128, FC // 128], _f32, kind="ExternalInput")
    B2 = nc.dram_tensor("B2", [128, OTILES], _f32, kind="ExternalInput")
    H1D = nc.dram_tensor("H1D", [128, 1], _f32, kind="ExternalInput")
    BHN = nc.dram_tensor("BHN", [128, 1], _f32, kind="ExternalInput")
    OT = nc.dram_tensor("OT", [OUT, BC], _f32, kind="ExternalOutput")

    ew_u = nc.gpsimd if GP_U else nc.vector
    ew_hp = nc.gpsimd if GP_HP else nc.vector
    ew_fr = nc.gpsimd if GP_FR else nc.vector

    with tile.TileContext(nc) as tc:
        with (
            tc.tile_pool(name="weights", bufs=1) as wp,
            tc.tile_pool(name="xin", bufs=10) as xp,
            tc.tile_pool(name="hstate", bufs=ND) as hp,
            tc.tile_pool(name="hfinal", bufs=1) as hf,
            tc.tile_pool(name="gates", bufs=5) as gp,
            tc.tile_pool(name="head", bufs=3) as fp,
            tc.tile_pool(name="frhs", bufs=1) as frp,
            tc.tile_pool(name="outs", bufs=3) as op_,
            tc.tile_pool(name="psA", bufs=2, space="PSUM") as psA,
            tc.tile_pool(name="psB", bufs=2, space="PSUM") as psB,
        ):
            # --- load weights/biases once (host pre-packs duo duplication) ---
            wx = wp.tile([128, 3 * H], dt)
            nc.sync.dma_start(out=wx[:], in_=WX[:])
            wh = wp.tile([128, 3 * H], dt)
            nc.sync.dma_start(out=wh[:], in_=WH[:])
            w1d = wp.tile([128, FC], dt)
            nc.sync.dma_start(out=w1d[:], in_=W1D[:])
            w2a = wp.tile([128, OUT], dt)
            nc.sync.dma_start(out=w2a[:], in_=W2T[0:128, :])
            w2b = wp.tile([128, OUT], dt)
            nc.sync.dma_start(out=w2b[:], in_=W2T[128:256, :])
            b1s = wp.tile([128, FC // 128], _f32)
            nc.sync.dma_start(out=b1s[:], in_=B1[:])
            b2s = wp.tile([128, OTILES], _f32)
            nc.sync.dma_start(out=b2s[:], in_=B2[:])
            h1d = wp.tile([128, 1], _f32)
            nc.sync.dma_start(out=h1d[:], in_=H1D[:])
            bhn = wp.tile([128, 1], _f32)
            nc.sync.dma_start(out=bhn[:], in_=BHN[:])

            # x tiles: ones rows (bias folding) preset once per pool buffer;
            # the per-step DMA only touches rows 0:KX and 64:64+KX.
            for _xi in range(10):
                xz = xp.tile([128, BLK], dt, tag="xt")
                nc.vector.memset(xz[:], 0.0)
                nc.vector.memset(xz[32:33, :], 1.0)
                nc.vector.memset(xz[96:97, :], 1.0)

            def start_duo(d):
                ha = hp.tile([128, BLK], dt, tag="ha")
                hb = hp.tile([128, BLK], dt, tag="hb")
                hfd = hf.tile([128, BLK], dt, tag=f"hf{d}")
                nc.vector.memset(ha[:], 0.0)
                nc.vector.tensor_scalar_add(ha[:], ha[:], h1d[:])
                if duo_steps[d] == 0:
                    nc.vector.memset(hfd[:], 0.0)
                    nc.vector.tensor_scalar_add(hfd[:], hfd[:], h1d[:])
                return {"cur": ha, "nxt": hb, "hf": hfd, "d": d}

            def emit_step(st, s):
                d = st["d"]
                cur = st["cur"]
                xt = xp.tile([128, BLK], dt, tag="xt")
                lo = d * DUO
                hi = lo + BLK
                nc.sync.dma_start(out=xt[0:KX, :], in_=X[s - 1, :, lo : lo + BLK])
                nc.sync.dma_start(
                    out=xt[64 : 64 + KX, :], in_=X[s - 1, :, hi : hi + BLK]
                )
                prz = psA.tile([128, DUO], _f32, tag="rz")
                pn = psB.tile([128, DUO], _f32, tag="n")
                # gate matmuls; weight cols r=0:64, z=64:128, n=128:192.
                # lo quadrant (rows 0:64 -> cols 0:64), hi (64:128 -> 64:128);
                # adjacent lo/hi matmuls execute concurrently.
                for gc, fds in ((0, slice(0, BLK)), (64, slice(BLK, DUO))):
                    # r (gc=0) / z (gc=64) preactivations -> prz[:, fds]
                    nc.tensor.matmul(
                        prz[0:64, fds], wh[0:64, gc : gc + 64], cur[0:64, :],
                        start=True, stop=False,
                    )
                    nc.tensor.matmul(
                        prz[64:128, fds], wh[64:128, gc : gc + 64], cur[64:128, :],
                        start=True, stop=False,
                    )
                    nc.tensor.matmul(
                        prz[0:64, fds], wx[0:KXR, gc : gc + 64], xt[0:KXR, :],
                        start=False, stop=True,
                    )
                    nc.tensor.matmul(
                        prz[64:128, fds], wx[64 : 64 + KXR, gc : gc + 64],
                        xt[64 : 64 + KXR, :], start=False, stop=True,
                    )
                # hn -> pn[:, 0:BLK] (h side only), xn -> pn[:, BLK:] (x side)
                nc.tensor.matmul(
                    pn[0:64, 0:BLK], wh[0:64, 128:192], cur[0:64, :],
                    start=True, stop=True,
                )
                nc.tensor.matmul(
                    pn[64:128, 0:BLK], wh[64:128, 128:192], cur[64:128, :],
                    start=True, stop=True,
                )
                nc.tensor.matmul(
                    pn[0:64, BLK:], wx[0:KXR, 128:192], xt[0:KXR, :],
                    start=True, stop=True,
                )
                nc.tensor.matmul(
                    pn[64:128, BLK:], wx[64 : 64 + KXR, 128:192],
                    xt[64 : 64 + KXR, :], start=True, stop=True,
                )
                # elementwise (all duo-width: 128 partitions, FD 512/1024)
                rz = gp.tile([128, DUO], dt, tag="rz")
                nc.scalar.activation(rz[:], prz[:], AF.Sigmoid)
                t1 = gp.tile([128, BLK], dt, tag="t1")
                nc.vector.scalar_tensor_tensor(
                    t1[:], pn[:, 0:BLK], bhn[:], rz[:, 0:BLK],
                    op0=OP.add, op1=OP.mult,
                )
                t2 = gp.tile([128, BLK], dt, tag="t2")
                nc.vector.tensor_add(t2[:], pn[:, BLK:], t1[:])
                nt = gp.tile([128, BLK], dt, tag="nt")
                nc.scalar.activation(nt[:], t2[:], AF.Tanh)
                u = gp.tile([128, BLK], dt, tag="u")
                ew_u.tensor_sub(u[:], cur[:], nt[:])
                v = gp.tile([128, BLK], dt, tag="v")
                nc.vector.tensor_mul(v[:], rz[:, BLK:], u[:])
                dst = st["hf"] if s == duo_steps[d] else st["nxt"]
                ew_hp.tensor_add(dst[:], nt[:], v[:])
                st["cur"], st["nxt"] = dst, st["cur"]

            def emit_mlp(g, hfin):
                """fc1 + ELU + fc2 + sigmoid for 2-duo group g."""
                d0, d1 = 2 * g, 2 * g + 1
                frs = {}
                for j in range(FC // 128):
                    for half, base in ((0, 0), (1, 64)):
                        pf = psA.tile([128, DUO], _f32, tag="rz")
                        for k, dd in enumerate((d0, d1)):
                            nc.tensor.matmul(
                                pf[:, k * BLK : (k + 1) * BLK],
                                w1d[base : base + 64, j * 128 : (j + 1) * 128],
                                hfin[dd][base : base + 64, :],
                                start=True, stop=True,
                            )
                        # elu(x+b1) + 1 == (x+b1 - m) + exp(m),  m = min(x+b1, 0)
                        m = fp.tile([128, DUO], dt, tag="m")
                        nc.vector.tensor_scalar(
                            m[:], pf[:], b1s[:, j : j + 1], 0.0,
                            op0=OP.add, op1=OP.min,
                        )
                        e = fp.tile([128, DUO], dt, tag="e")
                        nc.scalar.activation(e[:], m[:], AF.Exp)
                        pp = fp.tile([128, DUO], dt, tag="pp")
                        nc.vector.scalar_tensor_tensor(
                            pp[:], pf[:], b1s[:, j : j + 1], m[:],
                            op0=OP.add, op1=OP.subtract,
                        )
                        fr = frp.tile([128, DUO], dt, tag=f"fr{g}{j}{half}")
                        ew_fr.tensor_add(fr[:], pp[:], e[:])
                        frs[(j, half)] = fr
                for half in (0, 1):
                    for mt in range(OTILES):
                        mw = min(128, OUT - mt * 128)
                        po = psB.tile([128, DUO], _f32, tag="n")
                        for cs in (slice(0, BLK), slice(BLK, DUO)):
                            nc.tensor.matmul(
                                po[0:mw, cs], w2a[:, mt * 128 : mt * 128 + mw],
                                frs[(0, half)][:, cs], start=True, stop=False,
                            )
                            nc.tensor.matmul(
                                po[0:mw, cs], w2b[:, mt * 128 : mt * 128 + mw],
                                frs[(1, half)][:, cs], start=False, stop=True,
                            )
                        ot = op_.tile([mw, DUO], _f32, tag="ot")
                        nc.scalar.activation(
                            ot[:], po[0:mw, :], AF.Sigmoid,
                            bias=b2s[0:mw, mt : mt + 1],
                        )
                        # group g = blocks 4g..4g+3; half-lo = blocks 4g, 4g+2
                        b0 = (4 * g + half) * BLK
                        b1_ = (4 * g + 2 + half) * BLK
                        nc.sync.dma_start(
                            out=OT[mt * 128 : mt * 128 + mw, b0 : b0 + BLK],
                            in_=ot[:, 0:BLK],
                        )
                        nc.sync.dma_start(
                            out=OT[mt * 128 : mt * 128 + mw, b1_ : b1_ + BLK],
                            in_=ot[:, BLK:],
                        )

            states = [start_duo(d) for d in range(ND)]
            hfin = [st["hf"] for st in states]
            maxs = max(duo_steps)
            done_g = set()
            for s in range(1, maxs + 1):
                for st in states:
                    if s <= duo_steps[st["d"]]:
                        emit_step(st, s)
                if MLP_INLINE:
                    for g in range(ND // 2):
                        if g not in done_g and duo_steps[2 * g + 1] <= s:
                            emit_mlp(g, hfin)
                            done_g.add(g)
            if not MLP_INLINE:
                tc.no_sync_barrier()
            for g in range(ND // 2):
                if g not in done_g:
                    emit_mlp(g, hfin)

    nc.compile()
    return nc


def kernel(message, lengths, init_emb, W_emb, b_emb, W_ih, W_hh, b_ih, b_hh,
           W1, b1, W2, b2):
    global LAST_RESULT
    message = np.asarray(message, dtype=np.float32)
    lengths = np.asarray(lengths).astype(np.int64)
    f8 = np.float64
    np_dt = ml_dtypes.bfloat16 if USE_BF16 else np.float32
    dt = _bf16 if USE_BF16 else _f32

    # --- fold embedding into input weights;  step 0 is a constant ---
    W_c = W_ih.astype(f8) @ W_emb.astype(f8)                # [3H, V]
    b_c = W_ih.astype(f8) @ b_emb.astype(f8) + b_ih         # [3H]
    gx0 = W_ih.astype(f8) @ init_emb.astype(f8) + b_ih
    gh0 = b_hh.astype(f8)
    r0 = _sigmoid(gx0[:H] + gh0[:H])
    z0 = _sigmoid(gx0[H : 2 * H] + gh0[H : 2 * H])
    n0 = np.tanh(gx0[2 * H :] + r0 * gh0[2 * H :])
    h1 = (1.0 - z0) * n0                                    # h after step 0

    # --- length-sort, stratify across cores ---
    perm = np.argsort(lengths, kind="stable")
    lsort = lengths[perm]
    # duo d (on every core) covers global sorted ranks [d*8192, (d+1)*8192)
    duo_steps = [int(lsort[min((d + 1) * DUO * NCORES, B) - 1]) - 1
                 for d in range(ND)]

    # --- shared weight tensors (duo: both partition halves identical) ---
    brz = (b_c[: 2 * H] + b_hh[: 2 * H])
    WXh = np.zeros((64, 3 * H), f8)
    WXh[:V] = W_c.T
    WXh[V, H : 2 * H] = FREEZE
    WXh[32, : 2 * H] = brz
    WXh[32, 2 * H :] = b_c[2 * H :]
    wxd = np.concatenate([WXh, WXh]).astype(np_dt)
    whh = np.ascontiguousarray(W_hh.T).astype(f8)
    whd = np.concatenate([whh, whh]).astype(np_dt)
    w1h = np.ascontiguousarray(W1.T).astype(f8)
    w1dd = np.concatenate([w1h, w1h]).astype(np_dt)
    w2d = np.ascontiguousarray(W2.T).astype(np_dt)
    b1d = np.ascontiguousarray(np.asarray(b1, np.float32).reshape(FC // 128, 128).T)
    b2f = (np.asarray(b2, f8) - W2.astype(f8).sum(axis=1)).astype(np.float32)
    b2p = np.zeros(OTILES * 128, np.float32)
    b2p[:OUT] = b2f
    b2d = np.ascontiguousarray(b2p.reshape(OTILES, 128).T)
    h1d = np.ascontiguousarray(
        np.concatenate([h1, h1]).astype(np.float32).reshape(128, 1))
    bhnd = np.ascontiguousarray(
        np.concatenate([b_hh[2 * H :], b_hh[2 * H :]]).astype(np.float32)
        .reshape(128, 1))

    # --- per-core inputs ---
    trange = np.arange(T - 1)
    in_maps = []
    core_idx = []
    for c in range(NCORES):
        ic = perm[c::NCORES]
        core_idx.append(ic)
        mc = message[ic][:, : T - 1, :]                     # [BC, 29, 21]
        Xc = np.empty((T - 1, KX, BC), dtype=np_dt)
        Xc[:, :V, :] = mc.transpose(1, 2, 0).astype(np_dt)
        Xc[:, V, :] = (lengths[ic][None, :] <= trange[:, None] + 1).astype(np_dt)
        in_maps.append({
            "X": Xc, "WX": wxd, "WH": whd, "W1D": w1dd, "W2T": w2d,
            "B1": b1d, "B2": b2d, "H1D": h1d, "BHN": bhnd,
        })

    nc = _build_nc(duo_steps, dt)
    res = run_bass_kernel_spmd(nc, in_maps, core_ids=list(range(NCORES)), trace=TRACE)
    LAST_RESULT = res

    out = np.empty((B, OUT), np.float32)
    for c in range(NCORES):
        out[core_idx[c]] = res.results[c]["OT"].T
    return out


# revision 15
# speedup vs baseline: 1.1252x; 1.0173x over previous
"""Trainium2 Bass kernel for nn_Decoder (GRU decoder over padded sequences).

Computation (per sample):
  emb = message[:, :T-1] @ W_emb.T + b_emb            (folded into W_c on host)
  xs  = [init_emb, emb]                                (step 0 folded into h1 const)
  GRU over T steps, gather h at lengths-1              (freeze trick: z := 1 past len)
  out = sigmoid(elu(h @ W1.T + b1) @ W2.T + b2)

Sharding: batch data-parallel over 8 cores, host-side length-sort (stratified
round-robin across cores) so each duo only runs to its max length.

DUO layout: every on-chip tile stacks TWO 512-sample blocks on the partition
axis (block-lo on partitions 0:63, block-hi on 64:127).  Elementwise cost on
DVE/ACT scales with the free dim only, so running the GRU's 64-feature math
at 128 partitions halves the per-sample elementwise cost vs a [64, FD]
layout.  Matmuls use PE quadrant tiling: lo contracts array rows 0:63 into
cols 0:63, hi contracts rows 64:127 into cols 64:127 (weights duplicated in
both halves), so lo/hi matmuls execute concurrently in disjoint quadrants.
Gate biases ride ones-rows of the x tile; b_hn is applied by the DVE
scalar_tensor_tensor that forms r*(hn+b_hn).
"""

import sys

sys.path.insert(0, "/opt/trn_rl_repo")

import numpy as np
import ml_dtypes

import concourse.bacc as bacc
import concourse.mybir as mybir
import concourse.tile as tile
from concourse.bass_utils import run_bass_kernel_spmd

B, T, V, E, H, FC, OUT = 65536, 30, 21, 32, 64, 256, 784
NCORES = 8
BC = B // NCORES          # 8192 samples per core
BLK = 512                 # one sample block (one PSUM bank of fp32)
DUO = 2 * BLK             # samples per duo (lo block + hi block)
ND = BC // DUO            # 8 duos per core
FREEZE = 40.0             # z-gate preactivation offset for finished samples
KX = V + 1                # 21 msg rows + 1 freeze-flag row (DMA'd)
KXR = 33                  # x contraction: rows 0:22 data, 22:32 zero, 32 ones
OTILES = (OUT + 127) // 128  # 7 output row tiles

USE_BF16 = True
GP_U = True               # u = h - n on GpSimd (else VectorE)
GP_HP = False             # h' = n + v on GpSimd
GP_FR = True              # fr = pp + e on GpSimd
MLP_INLINE = False        # emit each 2-duo MLP group as soon as its duos finish
TRACE = False             # set by test harness for profiling
LAST_RESULT = None        # BassKernelResults stash for the harness

_f32 = mybir.dt.float32
_bf16 = mybir.dt.bfloat16


def _sigmoid(x):
    return 1.0 / (1.0 + np.exp(-x))


def _build_nc(duo_steps, dt):
    """Build the SPMD kernel. duo_steps[d] = GRU steps (beyond the constant
    step 0) for duo d — identical on every core."""
    AF = mybir.ActivationFunctionType
    OP = mybir.AluOpType
    nc = bacc.Bacc("TRN2", target_bir_lowering=False, debug=False)

    X = nc.dram_tensor("X", [T - 1, KX, BC], dt, kind="ExternalInput")
    WX = nc.dram_tensor("WX", [128, 3 * H], dt, kind="ExternalInput")
    WH = nc.dram_tensor("WH", [128, 3 * H], dt, kind="ExternalInput")
    W1D = nc.dram_tensor("W1D", [128, FC], dt, kind="ExternalInput")
    W2T = nc.dram_tensor("W2T", [FC, OUT], dt, kind="ExternalInput")
    B1 = nc.dram_tensor("B1", [128, FC // 128], _f32, kind="ExternalInput")
    B2 = nc.dram_tensor("B2", [128, OTILES], _f32, kind="ExternalInput")
    H1D = nc.dram_tensor("H1D", [128, 1], _f32, kind="ExternalInput")
    BHN = nc.dram_tensor("BHN", [128, 1], _f32, kind="ExternalInput")
    OT = nc.dram_tensor("OT", [OUT, BC], _f32, kind="ExternalOutput")

    ew_u = nc.gpsimd if GP_U else nc.vector
    ew_hp = nc.gpsimd if GP_HP else nc.vector
    ew_fr = nc.gpsimd if GP_FR else nc.vector

    with tile.TileContext(nc) as tc:
        with (
            tc.tile_pool(name="weights", bufs=1) as wp,
            tc.tile_pool(name="xin", bufs=10) as xp,
            tc.tile_pool(name="hstate", bufs=ND) as hp,
            tc.tile_pool(name="hfinal", bufs=1) as hf,
            tc.tile_pool(name="gates", bufs=5) as gp,
            tc.tile_pool(name="head", bufs=3) as fp,
            tc.tile_pool(name="frhs", bufs=1) as frp,
            tc.tile_pool(name="outs", bufs=3) as op_,
            tc.tile_pool(name="psA", bufs=2, space="PSUM") as psA,
            tc.tile_pool(name="psB", bufs=2, space="PSUM") as psB,
        ):
            # --- load weights/biases once (host pre-packs duo duplication) ---
            wx = wp.tile([128, 3 * H], dt)
            nc.sync.dma_start(out=wx[:], in_=WX[:])
            wh = wp.tile([128, 3 * H], dt)
            nc.sync.dma_start(out=wh[:], in_=WH[:])
            w1d = wp.tile([128, FC], dt)
            nc.sync.dma_start(out=w1d[:], in_=W1D[:])
            w2a = wp.tile([128, OUT], dt)
            nc.sync.dma_start(out=w2a[:], in_=W2T[0:128, :])
            w2b = wp.tile([128, OUT], dt)
            nc.sync.dma_start(out=w2b[:], in_=W2T[128:256, :])
            b1s = wp.tile([128, FC // 128], _f32)
            nc.sync.dma_start(out=b1s[:], in_=B1[:])
            b2s = wp.tile([128, OTILES], _f32)
            nc.sync.dma_start(out=b2s[:], in_=B2[:])
            h1d = wp.tile([128, 1], _f32)
            nc.sync.dma_start(out=h1d[:], in_=H1D[:])
            bhn = wp.tile([128, 1], _f32)
            nc.sync.dma_start(out=bhn[:], in_=BHN[:])

            # x tiles: ones rows (bias folding) preset once per pool buffer;
            # the per-step DMA only touches rows 0:KX and 64:64+KX.
            for _xi in range(10):
                xz = xp.tile([128, BLK], dt, tag="xt")
                nc.vector.memset(xz[:], 0.0)
                nc.vector.memset(xz[32:33, :], 1.0)
                nc.vector.memset(xz[96:97, :], 1.0)

            def start_duo(d):
                ha = hp.tile([128, BLK], dt, tag="ha")
                hb = hp.tile([128, BLK], dt, tag="hb")
                hfd = hf.tile([128, BLK], dt, tag=f"hf{d}")
                nc.vector.memset(ha[:], 0.0)
                nc.vector.tensor_scalar_add(ha[:], ha[:], h1d[:])
                if duo_steps[d] == 0:
                    nc.vector.memset(hfd[:], 0.0)
                    nc.vector.tensor_scalar_add(hfd[:], hfd[:], h1d[:])
                return {"cur": ha, "nxt": hb, "hf": hfd, "d": d}

            def emit_step(st, s):
                d = st["d"]
                cur = st["cur"]
                xt = xp.tile([128, BLK], dt, tag="xt")
                lo = d * DUO
                hi = lo + BLK
                nc.sync.dma_start(out=xt[0:KX, :], in_=X[s - 1, :, lo : lo + BLK])
                nc.sync.dma_start(
                    out=xt[64 : 64 + KX, :], in_=X[s - 1, :, hi : hi + BLK]
                )
                prz = psA.tile([128, DUO], _f32, tag="rz")
                pn = psB.tile([128, DUO], _f32, tag="n")
                # gate matmuls; weight cols r=0:64, z=64:128, n=128:192.
                # lo quadrant (rows 0:64 -> cols 0:64), hi (64:128 -> 64:128);
                # adjacent lo/hi matmuls execute concurrently.
                for gc, fds in ((0, slice(0, BLK)), (64, slice(BLK, DUO))):
                    # r (gc=0) / z (gc=64) preactivations -> prz[:, fds]
                    nc.tensor.matmul(
                        prz[0:64, fds], wh[0:64, gc : gc + 64], cur[0:64, :],
                        start=True, stop=False,
                    )
                    nc.tensor.matmul(
                        prz[64:128, fds], wh[64:128, gc : gc + 64], cur[64:128, :],
                        start=True, stop=False,
                    )
                    nc.tensor.matmul(
                        prz[0:64, fds], wx[0:KXR, gc : gc + 64], xt[0:KXR, :],
                        start=False, stop=True,
                    )
                    nc.tensor.matmul(
                        prz[64:128, fds], wx[64 : 64 + KXR, gc : gc + 64],
                        xt[64 : 64 + KXR, :], start=False, stop=True,
                    )
                # hn -> pn[:, 0:BLK] (h side only), xn -> pn[:, BLK:] (x side)
                nc.tensor.matmul(
                    pn[0:64, 0:BLK], wh[0:64, 128:192], cur[0:64, :],
                    start=True, stop=True,
                )
                nc.tensor.matmul(
                    pn[64:128, 0:BLK], wh[64:128, 128:192], cur[64:128, :],
                    start=True, stop=True,
                )
                nc.tensor.matmul(
                    pn[0:64, BLK:], wx[0:KXR, 128:192], xt[0:KXR, :],
                    start=True, stop=True,
                )
                nc.tensor.matmul(
                    pn[64:128, BLK:], wx[64 : 64 + KXR, 128:192],
                    xt[64 : 64 + KXR, :], start=True, stop=True,
                )
                # elementwise (all duo-width: 128 partitions, FD 512/1024)
                rz = gp.tile([128, DUO], dt, tag="rz")
                nc.scalar.activation(rz[:], prz[:], AF.Sigmoid)
                t1 = gp.tile([128, BLK], dt, tag="t1")
                nc.vector.scalar_tensor_tensor(
                    t1[:], pn[:, 0:BLK], bhn[:], rz[:, 0:BLK],
                    op0=OP.add, op1=OP.mult,
                )
                t2 = gp.tile([128, BLK], dt, tag="t2")
                nc.vector.tensor_add(t2[:], pn[:, BLK:], t1[:])
                nt = gp.tile([128, BLK], dt, tag="nt")
                nc.scalar.activation(nt[:], t2[:], AF.Tanh)
                u = gp.tile([128, BLK], dt, tag="u")
                ew_u.tensor_sub(u[:], cur[:], nt[:])
                v = gp.tile([128, BLK], dt, tag="v")
                nc.vector.tensor_mul(v[:], rz[:, BLK:], u[:])
                dst = st["hf"] if s == duo_steps[d] else st["nxt"]
                ew_hp.tensor_add(dst[:], nt[:], v[:])
                st["cur"], st["nxt"] = dst, st["cur"]

            def emit_mlp(g, hfin):
                """fc1 + ELU + fc2 + sigmoid for 2-duo group g.  PSUM tiles
                alternate between both pools (free post-GRU) for depth."""
                d0, d1 = 2 * g, 2 * g + 1
                frs = {}
                pcnt = [0]

                def ptile():
                    pool, tag = ((psA, "rz"), (psB, "n"))[pcnt[0] % 2]
                    pcnt[0] += 1
                    return pool.tile([128, DUO], _f32, tag=tag,
                                     name=f"pm{g}_{pcnt[0]}")

                for j in range(FC // 128):
                    for half, base in ((0, 0), (1, 64)):
                        pf = ptile()
                        for k, dd in enumerate((d0, d1)):
                            nc.tensor.matmul(
                                pf[:, k * BLK : (k + 1) * BLK],
                                w1d[base : base + 64, j * 128 : (j + 1) * 128],
                                hfin[dd][base : base + 64, :],
                                start=True, stop=True,
                            )
                        # elu(x+b1) + 1 == (x+b1 - m) + exp(m),  m = min(x+b1, 0)
                        m = fp.tile([128, DUO], dt, tag="m")
                        nc.vector.tensor_scalar(
                            m[:], pf[:], b1s[:, j : j + 1], 0.0,
                            op0=OP.add, op1=OP.min,
                        )
                        e = fp.tile([128, DUO], dt, tag="e")
                        nc.scalar.activation(e[:], m[:], AF.Exp)
                        pp = fp.tile([128, DUO], dt, tag="pp")
                        nc.vector.scalar_tensor_tensor(
                            pp[:], pf[:], b1s[:, j : j + 1], m[:],
                            op0=OP.add, op1=OP.subtract,
                        )
                        fr = frp.tile([128, DUO], dt, tag=f"fr{g}{j}{half}")
                        ew_fr.tensor_add(fr[:], pp[:], e[:])
                        frs[(j, half)] = fr
                for half in (0, 1):
                    for mt in range(OTILES):
                        mw = min(128, OUT - mt * 128)
                        po = ptile()
                        for cs in (slice(0, BLK), slice(BLK, DUO)):
                            nc.tensor.matmul(
                                po[0:mw, cs], w2a[:, mt * 128 : mt * 128 + mw],
                                frs[(0, half)][:, cs], start=True, stop=False,
                            )
                            nc.tensor.matmul(
                                po[0:mw, cs], w2b[:, mt * 128 : mt * 128 + mw],
                                frs[(1, half)][:, cs], start=False, stop=True,
                            )
                        ot = op_.tile([mw, DUO], _f32, tag="ot")
                        nc.scalar.activation(
                            ot[:], po[0:mw, :], AF.Sigmoid,
                            bias=b2s[0:mw, mt : mt + 1],
                        )
                        # group g = blocks 4g..4g+3; half-lo = blocks 4g, 4g+2
                        b0 = (4 * g + half) * BLK
                        b1_ = (4 * g + 2 + half) * BLK
                        nc.sync.dma_start(
                            out=OT[mt * 128 : mt * 128 + mw, b0 : b0 + BLK],
                            in_=ot[:, 0:BLK],
                        )
                        nc.sync.dma_start(
                            out=OT[mt * 128 : mt * 128 + mw, b1_ : b1_ + BLK],
                            in_=ot[:, BLK:],
                        )

            states = [start_duo(d) for d in range(ND)]
            hfin = [st["hf"] for st in states]
            maxs = max(duo_steps)
            done_g = set()
            for s in range(1, maxs + 1):
                for st in states:
                    if s <= duo_steps[st["d"]]:
                        emit_step(st, s)
                if MLP_INLINE:
                    for g in range(ND // 2):
                        if g not in done_g and duo_steps[2 * g + 1] <= s:
                            emit_mlp(g, hfin)
                            done_g.add(g)
            if not MLP_INLINE:
                tc.no_sync_barrier()
            for g in range(ND // 2):
                if g not in done_g:
                    emit_mlp(g, hfin)

    nc.compile()
    return nc


def kernel(message, lengths, init_emb, W_emb, b_emb, W_ih, W_hh, b_ih, b_hh,
           W1, b1, W2, b2):
    global LAST_RESULT
    message = np.asarray(message, dtype=np.float32)
    lengths = np.asarray(lengths).astype(np.int64)
    f8 = np.float64
    np_dt = ml_dtypes.bfloat16 if USE_BF16 else np.float32
    dt = _bf16 if USE_BF16 else _f32

    # --- fold embedding into input weights;  step 0 is a constant ---
    W_c = W_ih.astype(f8) @ W_emb.astype(f8)                # [3H, V]
    b_c = W_ih.astype(f8) @ b_emb.astype(f8) + b_ih         # [3H]
    gx0 = W_ih.astype(f8) @ init_emb.astype(f8) + b_ih
    gh0 = b_hh.astype(f8)
    r0 = _sigmoid(gx0[:H] + gh0[:H])
    z0 = _sigmoid(gx0[H : 2 * H] + gh0[H : 2 * H])
    n0 = np.tanh(gx0[2 * H :] + r0 * gh0[2 * H :])
    h1 = (1.0 - z0) * n0                                    # h after step 0

    # --- length-sort, stratify across cores ---
    perm = np.argsort(lengths, kind="stable")
    lsort = lengths[perm]
    # duo d (on every core) covers global sorted ranks [d*8192, (d+1)*8192)
    duo_steps = [int(lsort[min((d + 1) * DUO * NCORES, B) - 1]) - 1
                 for d in range(ND)]

    # --- shared weight tensors (duo: both partition halves identical) ---
    brz = (b_c[: 2 * H] + b_hh[: 2 * H])
    WXh = np.zeros((64, 3 * H), f8)
    WXh[:V] = W_c.T
    WXh[V, H : 2 * H] = FREEZE
    WXh[32, : 2 * H] = brz
    WXh[32, 2 * H :] = b_c[2 * H :]
    wxd = np.concatenate([WXh, WXh]).astype(np_dt)
    whh = np.ascontiguousarray(W_hh.T).astype(f8)
    whd = np.concatenate([whh, whh]).astype(np_dt)
    w1h = np.ascontiguousarray(W1.T).astype(f8)
    w1dd = np.concatenate([w1h, w1h]).astype(np_dt)
    w2d = np.ascontiguousarray(W2.T).astype(np_dt)
    b1d = np.ascontiguousarray(np.asarray(b1, np.float32).reshape(FC // 128, 128).T)
    b2f = (np.asarray(b2, f8) - W2.astype(f8).sum(axis=1)).astype(np.float32)
    b2p = np.zeros(OTILES * 128, np.float32)
    b2p[:OUT] = b2f
    b2d = np.ascontiguousarray(b2p.reshape(OTILES, 128).T)
    h1d = np.ascontiguousarray(
        np.concatenate([h1, h1]).astype(np.float32).reshape(128, 1))
    bhnd = np.ascontiguousarray(
        np.concatenate([b_hh[2 * H :], b_hh[2 * H :]]).astype(np.float32)
        .reshape(128, 1))

    # --- per-core inputs ---
    trange = np.arange(T - 1)
    in_maps = []
    core_idx = []
    for c in range(NCORES):
        ic = perm[c::NCORES]
        core_idx.append(ic)
        mc = message[ic][:, : T - 1, :]                     # [BC, 29, 21]
        Xc = np.empty((T - 1, KX, BC), dtype=np_dt)
        Xc[:, :V, :] = mc.transpose(1, 2, 0).astype(np_dt)
        Xc[:, V, :] = (lengths[ic][None, :] <= trange[:, None] + 1).astype(np_dt)
        in_maps.append({
            "X": Xc, "WX": wxd, "WH": whd, "W1D": w1dd, "W2T": w2d,
            "B1": b1d, "B2": b2d, "H1D": h1d, "BHN": bhnd,
        })

    nc = _build_nc(duo_steps, dt)
    res = run_bass_kernel_spmd(nc, in_maps, core_ids=list(range(NCORES)), trace=TRACE)
    LAST_RESULT = res

    out = np.empty((B, OUT), np.float32)
    for c in range(NCORES):
        out[core_idx[c]] = res.results[c]["OT"].T
    return out


# revision 16
# speedup vs baseline: 1.1633x; 1.0339x over previous
"""Trainium2 Bass kernel for nn_Decoder (GRU decoder over padded sequences).

Computation (per sample):
  emb = message[:, :T-1] @ W_emb.T + b_emb            (folded into W_c on host)
  xs  = [init_emb, emb]                                (step 0 folded into h1 const)
  GRU over T steps, gather h at lengths-1              (freeze trick: z := 1 past len)
  out = sigmoid(elu(h @ W1.T + b1) @ W2.T + b2)

Sharding: batch data-parallel over 8 cores, host-side length-sort (stratified
round-robin across cores) so each duo only runs to its max length.

DUO layout: every on-chip tile stacks TWO 512-sample blocks on the partition
axis (block-lo on partitions 0:63, block-hi on 64:127).  Elementwise cost on
DVE/ACT scales with the free dim only, so running the GRU's 64-feature math
at 128 partitions halves the per-sample elementwise cost vs a [64, FD]
layout.  Matmuls use PE quadrant tiling: lo contracts array rows 0:63 into
cols 0:63, hi contracts rows 64:127 into cols 64:127 (weights duplicated in
both halves), so lo/hi matmuls execute concurrently in disjoint quadrants.
Gate biases ride ones-rows of the x tile; b_hn is applied by the DVE
scalar_tensor_tensor that forms r*(hn+b_hn).
"""

import sys

sys.path.insert(0, "/opt/trn_rl_repo")

import numpy as np
import ml_dtypes

import concourse.bacc as bacc
import concourse.mybir as mybir
import concourse.tile as tile
from concourse.bass_utils import run_bass_kernel_spmd

B, T, V, E, H, FC, OUT = 65536, 30, 21, 32, 64, 256, 784
NCORES = 8
BC = B // NCORES          # 8192 samples per core
BLK = 512                 # one sample block (one PSUM bank of fp32)
DUO = 2 * BLK             # samples per duo (lo block + hi block)
ND = BC // DUO            # 8 duos per core
FREEZE = 40.0             # z-gate preactivation offset for finished samples
KX = V + 1                # 21 msg rows + 1 freeze-flag row (DMA'd)
KXR = 33                  # x contraction: rows 0:22 data, 22:32 zero, 32 ones
OTILES = (OUT + 127) // 128  # 7 output row tiles

USE_BF16 = True
GP_U = False              # u = h - n on GpSimd (else VectorE)
GP_HP = False             # h' = n + v on GpSimd
GP_FR = False             # fr = pp + e on GpSimd
MLP_INLINE = False        # emit each 2-duo MLP group as soon as its duos finish
TRACE = False             # set by test harness for profiling
LAST_RESULT = None        # BassKernelResults stash for the harness

_f32 = mybir.dt.float32
_bf16 = mybir.dt.bfloat16


def _sigmoid(x):
    return 1.0 / (1.0 + np.exp(-x))


def _build_nc(duo_steps, dt):
    """Build the SPMD kernel. duo_steps[d] = GRU steps (beyond the constant
    step 0) for duo d — identical on every core."""
    AF = mybir.ActivationFunctionType
    OP = mybir.AluOpType
    nc = bacc.Bacc("TRN2", target_bir_lowering=False, debug=False)

    X = nc.dram_tensor("X", [T - 1, KX, BC], dt, kind="ExternalInput")
    WX = nc.dram_tensor("WX", [128, 3 * H], dt, kind="ExternalInput")
    WH = nc.dram_tensor("WH", [128, 3 * H], dt, kind="ExternalInput")
    W1D = nc.dram_tensor("W1D", [128, FC], dt, kind="ExternalInput")
    W2T = nc.dram_tensor("W2T", [FC, OUT], dt, kind="ExternalInput")
    B1 = nc.dram_tensor("B1", [128, FC // 128], _f32, kind="ExternalInput")
    B2 = nc.dram_tensor("B2", [128, OTILES], _f32, kind="ExternalInput")
    H1D = nc.dram_tensor("H1D", [128, 1], _f32, kind="ExternalInput")
    BHN = nc.dram_tensor("BHN", [128, 1], _f32, kind="ExternalInput")
    OT = nc.dram_tensor("OT", [OUT, BC], _f32, kind="ExternalOutput")

    ew_u = nc.gpsimd if GP_U else nc.vector
    ew_hp = nc.gpsimd if GP_HP else nc.vector
    ew_fr = nc.gpsimd if GP_FR else nc.vector

    with tile.TileContext(nc) as tc:
        with (
            tc.tile_pool(name="weights", bufs=1) as wp,
            tc.tile_pool(name="xin", bufs=10) as xp,
            tc.tile_pool(name="hstate", bufs=ND) as hp,
            tc.tile_pool(name="hfinal", bufs=1) as hf,
            tc.tile_pool(name="gates", bufs=5) as gp,
            tc.tile_pool(name="head", bufs=3) as fp,
            tc.tile_pool(name="frhs", bufs=1) as frp,
            tc.tile_pool(name="outs", bufs=3) as op_,
            tc.tile_pool(name="psA", bufs=2, space="PSUM") as psA,
            tc.tile_pool(name="psB", bufs=2, space="PSUM") as psB,
        ):
            # --- load weights/biases once (host pre-packs duo duplication) ---
            wx = wp.tile([128, 3 * H], dt)
            nc.sync.dma_start(out=wx[:], in_=WX[:])
            wh = wp.tile([128, 3 * H], dt)
            nc.sync.dma_start(out=wh[:], in_=WH[:])
            w1d = wp.tile([128, FC], dt)
            nc.sync.dma_start(out=w1d[:], in_=W1D[:])
            w2a = wp.tile([128, OUT], dt)
            nc.sync.dma_start(out=w2a[:], in_=W2T[0:128, :])
            w2b = wp.tile([128, OUT], dt)
            nc.sync.dma_start(out=w2b[:], in_=W2T[128:256, :])
            b1s = wp.tile([128, FC // 128], _f32)
            nc.sync.dma_start(out=b1s[:], in_=B1[:])
            b2s = wp.tile([128, OTILES], _f32)
            nc.sync.dma_start(out=b2s[:], in_=B2[:])
            h1d = wp.tile([128, 1], _f32)
            nc.sync.dma_start(out=h1d[:], in_=H1D[:])
            bhn = wp.tile([128, 1], _f32)
            nc.sync.dma_start(out=bhn[:], in_=BHN[:])

            # x tiles: ones rows (bias folding) preset once per pool buffer;
            # the per-step DMA only touches rows 0:KX and 64:64+KX.
            for _xi in range(10):
                xz = xp.tile([128, BLK], dt, tag="xt")
                nc.vector.memset(xz[:], 0.0)
                nc.vector.memset(xz[32:33, :], 1.0)
                nc.vector.memset(xz[96:97, :], 1.0)

            def start_duo(d):
                ha = hp.tile([128, BLK], dt, tag="ha")
                hb = hp.tile([128, BLK], dt, tag="hb")
                hfd = hf.tile([128, BLK], dt, tag=f"hf{d}")
                nc.vector.memset(ha[:], 0.0)
                nc.vector.tensor_scalar_add(ha[:], ha[:], h1d[:])
                if duo_steps[d] == 0:
                    nc.vector.memset(hfd[:], 0.0)
                    nc.vector.tensor_scalar_add(hfd[:], hfd[:], h1d[:])
                return {"cur": ha, "nxt": hb, "hf": hfd, "d": d}

            def emit_step(st, s):
                d = st["d"]
                cur = st["cur"]
                xt = xp.tile([128, BLK], dt, tag="xt")
                lo = d * DUO
                hi = lo + BLK
                nc.sync.dma_start(out=xt[0:KX, :], in_=X[s - 1, :, lo : lo + BLK])
                nc.sync.dma_start(
                    out=xt[64 : 64 + KX, :], in_=X[s - 1, :, hi : hi + BLK]
                )
                prz = psA.tile([128, DUO], _f32, tag="rz")
                pn = psB.tile([128, DUO], _f32, tag="n")
                # gate matmuls; weight cols r=0:64, z=64:128, n=128:192.
                # lo quadrant (rows 0:64 -> cols 0:64), hi (64:128 -> 64:128);
                # adjacent lo/hi matmuls execute concurrently.
                for gc, fds in ((0, slice(0, BLK)), (64, slice(BLK, DUO))):
                    # r (gc=0) / z (gc=64) preactivations -> prz[:, fds]
                    nc.tensor.matmul(
                        prz[0:64, fds], wh[0:64, gc : gc + 64], cur[0:64, :],
                        start=True, stop=False,
                    )
                    nc.tensor.matmul(
                        prz[64:128, fds], wh[64:128, gc : gc + 64], cur[64:128, :],
                        start=True, stop=False,
                    )
                    nc.tensor.matmul(
                        prz[0:64, fds], wx[0:KXR, gc : gc + 64], xt[0:KXR, :],
                        start=False, stop=True,
                    )
                    nc.tensor.matmul(
                        prz[64:128, fds], wx[64 : 64 + KXR, gc : gc + 64],
                        xt[64 : 64 + KXR, :], start=False, stop=True,
                    )
                # hn -> pn[:, 0:BLK] (h side only), xn -> pn[:, BLK:] (x side)
                nc.tensor.matmul(
                    pn[0:64, 0:BLK], wh[0:64, 128:192], cur[0:64, :],
                    start=True, stop=True,
                )
                nc.tensor.matmul(
                    pn[64:128, 0:BLK], wh[64:128, 128:192], cur[64:128, :],
                    start=True, stop=True,
                )
                nc.tensor.matmul(
                    pn[0:64, BLK:], wx[0:KXR, 128:192], xt[0:KXR, :],
                    start=True, stop=True,
                )
                nc.tensor.matmul(
                    pn[64:128, BLK:], wx[64 : 64 + KXR, 128:192],
                    xt[64 : 64 + KXR, :], start=True, stop=True,
                )
                # elementwise (all duo-width: 128 partitions, FD 512/1024)
                rz = gp.tile([128, DUO], dt, tag="rz")
                nc.scalar.activation(rz[:], prz[:], AF.Sigmoid)
                t1 = gp.tile([128, BLK], dt, tag="t1")
                nc.vector.scalar_tensor_tensor(
                    t1[:], pn[:, 0:BLK], bhn[:], rz[:, 0:BLK],
                    op0=OP.add, op1=OP.mult,
                )
                t2 = gp.tile([128, BLK], dt, tag="t2")
                nc.vector.tensor_add(t2[:], pn[:, BLK:], t1[:])
                nt = gp.tile([128, BLK], dt, tag="nt")
                nc.scalar.activation(nt[:], t2[:], AF.Tanh)
                u = gp.tile([128, BLK], dt, tag="u")
                ew_u.tensor_sub(u[:], cur[:], nt[:])
                v = gp.tile([128, BLK], dt, tag="v")
                nc.vector.tensor_mul(v[:], rz[:, BLK:], u[:])
                dst = st["hf"] if s == duo_steps[d] else st["nxt"]
                ew_hp.tensor_add(dst[:], nt[:], v[:])
                st["cur"], st["nxt"] = dst, st["cur"]

            def emit_mlp(g, hfin):
                """fc1 + ELU + fc2 + sigmoid for 2-duo group g.  PSUM tiles
                alternate between both pools (free post-GRU) for depth."""
                d0, d1 = 2 * g, 2 * g + 1
                frs = {}
                pcnt = [0]

                def ptile():
                    pool, tag = ((psA, "rz"), (psB, "n"))[pcnt[0] % 2]
                    pcnt[0] += 1
                    return pool.tile([128, DUO], _f32, tag=tag,
                                     name=f"pm{g}_{pcnt[0]}")

                for j in range(FC // 128):
                    for half, base in ((0, 0), (1, 64)):
                        pf = ptile()
                        for k, dd in enumerate((d0, d1)):
                            nc.tensor.matmul(
                                pf[:, k * BLK : (k + 1) * BLK],
                                w1d[base : base + 64, j * 128 : (j + 1) * 128],
                                hfin[dd][base : base + 64, :],
                                start=True, stop=True,
                            )
                        # elu(x+b1) + 1 == (x+b1 - m) + exp(m),  m = min(x+b1, 0)
                        m = fp.tile([128, DUO], dt, tag="m")
                        nc.vector.tensor_scalar(
                            m[:], pf[:], b1s[:, j : j + 1], 0.0,
                            op0=OP.add, op1=OP.min,
                        )
                        e = fp.tile([128, DUO], dt, tag="e")
                        nc.scalar.activation(e[:], m[:], AF.Exp)
                        pp = fp.tile([128, DUO], dt, tag="pp")
                        nc.vector.scalar_tensor_tensor(
                            pp[:], pf[:], b1s[:, j : j + 1], m[:],
                            op0=OP.add, op1=OP.subtract,
                        )
                        fr = frp.tile([128, DUO], dt, tag=f"fr{g}{j}{half}")
                        ew_fr.tensor_add(fr[:], pp[:], e[:])
                        frs[(j, half)] = fr
                for half in (0, 1):
                    for mt in range(OTILES):
                        mw = min(128, OUT - mt * 128)
                        po = ptile()
                        for cs in (slice(0, BLK), slice(BLK, DUO)):
                            nc.tensor.matmul(
                                po[0:mw, cs], w2a[:, mt * 128 : mt * 128 + mw],
                                frs[(0, half)][:, cs], start=True, stop=False,
                            )
                            nc.tensor.matmul(
                                po[0:mw, cs], w2b[:, mt * 128 : mt * 128 + mw],
                                frs[(1, half)][:, cs], start=False, stop=True,
                            )
                        ot = op_.tile([mw, DUO], _f32, tag="ot")
                        nc.scalar.activation(
                            ot[:], po[0:mw, :], AF.Sigmoid,
                            bias=b2s[0:mw, mt : mt + 1],
                        )
                        # group g = blocks 4g..4g+3; half-lo = blocks 4g, 4g+2
                        b0 = (4 * g + half) * BLK
                        b1_ = (4 * g + 2 + half) * BLK
                        nc.sync.dma_start(
                            out=OT[mt * 128 : mt * 128 + mw, b0 : b0 + BLK],
                            in_=ot[:, 0:BLK],
                        )
                        nc.sync.dma_start(
                            out=OT[mt * 128 : mt * 128 + mw, b1_ : b1_ + BLK],
                            in_=ot[:, BLK:],
                        )

            states = [start_duo(d) for d in range(ND)]
            hfin = [st["hf"] for st in states]
            maxs = max(duo_steps)
            done_g = set()
            for s in range(1, maxs + 1):
                for st in states:
                    if s <= duo_steps[st["d"]]:
                        emit_step(st, s)
                if MLP_INLINE:
                    for g in range(ND // 2):
                        if g not in done_g and duo_steps[2 * g + 1] <= s:
                            emit_mlp(g, hfin)
                            done_g.add(g)
            if not MLP_INLINE:
                tc.no_sync_barrier()
            for g in range(ND // 2):
                if g not in done_g:
                    emit_mlp(g, hfin)

    nc.compile()
    return nc


def kernel(message, lengths, init_emb, W_emb, b_emb, W_ih, W_hh, b_ih, b_hh,
           W1, b1, W2, b2):
    global LAST_RESULT
    message = np.asarray(message, dtype=np.float32)
    lengths = np.asarray(lengths).astype(np.int64)
    f8 = np.float64
    np_dt = ml_dtypes.bfloat16 if USE_BF16 else np.float32
    dt = _bf16 if USE_BF16 else _f32

    # --- fold embedding into input weights;  step 0 is a constant ---
    W_c = W_ih.astype(f8) @ W_emb.astype(f8)                # [3H, V]
    b_c = W_ih.astype(f8) @ b_emb.astype(f8) + b_ih         # [3H]
    gx0 = W_ih.astype(f8) @ init_emb.astype(f8) + b_ih
    gh0 = b_hh.astype(f8)
    r0 = _sigmoid(gx0[:H] + gh0[:H])
    z0 = _sigmoid(gx0[H : 2 * H] + gh0[H : 2 * H])
    n0 = np.tanh(gx0[2 * H :] + r0 * gh0[2 * H :])
    h1 = (1.0 - z0) * n0                                    # h after step 0

    # --- length-sort, stratify across cores ---
    perm = np.argsort(lengths, kind="stable")
    lsort = lengths[perm]
    # duo d (on every core) covers global sorted ranks [d*8192, (d+1)*8192)
    duo_steps = [int(lsort[min((d + 1) * DUO * NCORES, B) - 1]) - 1
                 for d in range(ND)]

    # --- shared weight tensors (duo: both partition halves identical) ---
    brz = (b_c[: 2 * H] + b_hh[: 2 * H])
    WXh = np.zeros((64, 3 * H), f8)
    WXh[:V] = W_c.T
    WXh[V, H : 2 * H] = FREEZE
    WXh[32, : 2 * H] = brz
    WXh[32, 2 * H :] = b_c[2 * H :]
    wxd = np.concatenate([WXh, WXh]).astype(np_dt)
    whh = np.ascontiguousarray(W_hh.T).astype(f8)
    whd = np.concatenate([whh, whh]).astype(np_dt)
    w1h = np.ascontiguousarray(W1.T).astype(f8)
    w1dd = np.concatenate([w1h, w1h]).astype(np_dt)
    w2d = np.ascontiguousarray(W2.T).astype(np_dt)
    b1d = np.ascontiguousarray(np.asarray(b1, np.float32).reshape(FC // 128, 128).T)
    b2f = (np.asarray(b2, f8) - W2.astype(f8).sum(axis=1)).astype(np.float32)
    b2p = np.zeros(OTILES * 128, np.float32)
    b2p[:OUT] = b2f
    b2d = np.ascontiguousarray(b2p.reshape(OTILES, 128).T)
    h1d = np.ascontiguousarray(
        np.concatenate([h1, h1]).astype(np.float32).reshape(128, 1))
    bhnd = np.ascontiguousarray(
        np.concatenate([b_hh[2 * H :], b_hh[2 * H :]]).astype(np.float32)
        .reshape(128, 1))

    # --- per-core inputs ---
    trange = np.arange(T - 1)
    in_maps = []
    core_idx = []
    for c in range(NCORES):
        ic = perm[c::NCORES]
        core_idx.append(ic)
        mc = message[ic][:, : T - 1, :]                     # [BC, 29, 21]
        Xc = np.empty((T - 1, KX, BC), dtype=np_dt)
        Xc[:, :V, :] = mc.transpose(1, 2, 0).astype(np_dt)
        Xc[:, V, :] = (lengths[ic][None, :] <= trange[:, None] + 1).astype(np_dt)
        in_maps.append({
            "X": Xc, "WX": wxd, "WH": whd, "W1D": w1dd, "W2T": w2d,
            "B1": b1d, "B2": b2d, "H1D": h1d, "BHN": bhnd,
        })

    nc = _build_nc(duo_steps, dt)
    res = run_bass_kernel_spmd(nc, in_maps, core_ids=list(range(NCORES)), trace=TRACE)
    LAST_RESULT = res

    out = np.empty((B, OUT), np.float32)
    for c in range(NCORES):
        out[core_idx[c]] = res.results[c]["OT"].T
    return out
